# revision 18
# baseline (speedup 1.0000x reference)
"""nn_MultiHeadAttention sparse-attention kernel for 8 TRN2 NeuronCores.

Strategy: batch-parallel (B=8 -> 1 batch per core). Per-(i,j) bias terms:

  score2[h,i,j] = q[h,i,:].att_tab[tb[i,j]]        (+ -30*mask)
  out2[h,i,d]   = sum_j attn[h,i,j]*vec_tab[tb[i,j],d]

Both use host-expanded fp8 pair-tensors so TWO i values share each 128-column
PE weight load (weight loads, not FLOPs, bound these phases on HW):
 - score2: K-packed sum/diff trick. lhsT rows = [at[tb[i0,j]] | at[tb[i1,j]]],
   rhs cols = [q_i0;q_i1]/2 and [q_i0;-q_i1]/2 -> psum holds (s0+s1)/2 and
   (s0-s1)/2; a DVE add/sub pair recovers s0, s1. Mask is applied later as a
   bf16 identity-matmul accumulate (mbias) into the qk psum.
 - out2: M-packed: lhsT cols = [vt[tb[i0,j]] | vt[tb[i1,j]]], rhs = attn
   column pairs (N=16); valid halves are scattered out by two DVE copies.
Softmax denominators come free from a ones-column in the v weight tiles.
"""
import sys
import numpy as np

sys.path.insert(0, "/opt/trn_rl_repo")

HEADS = 8
B, L, HID = 8, 512, 512
D = HID // HEADS
NB = 183
MASK_NEG = -30.0

_NC_CACHE = {}


# ---------------------------------------------------------------- bass build
def _get_mods():
    import concourse.bass as bass
    import concourse.bacc as bacc
    import concourse.mybir as mybir
    import concourse.tile as tile
    return bass, bacc, mybir, tile


def split_excess_waits(nc, mybir):
    """This container's walrus supports only 1 sync wait on TPB_CTRL
    instructions (Drain/NoOp); split extras onto preceding 1-wait NoOps."""
    limited = ("Drain", "NoOp", "AllEngineBarrier", "Halt")
    for f in nc.m.functions:
        for bb in f.blocks:
            new_insts = []
            for inst in bb.instructions:
                si = inst.sync_info
                if (inst.opcode in limited and si is not None and si.on_wait
                        and len(si.on_wait) > 1):
                    waits = list(si.on_wait)
                    keep, extra = waits[:1], waits[1:]
                    eng = nc.engines[inst.engine]
                    for w in extra:
                        nop = eng.nop(hint="waitsplit", nofuse=True)
                        nopinst = nop.ins
                        for fb in nc.m.functions:
                            for bb2 in fb.blocks:
                                if nopinst in bb2.instructions:
                                    bb2.instructions.remove(nopinst)
                        nopinst.sync_info = mybir.SyncInfo(on_wait=[w], on_update=[])
                        new_insts.append(nopinst)
                    si.on_wait = keep
                new_insts.append(inst)
            bb.instructions[:] = new_insts


def build_kernel(loop_iters=0):
    """One-core kernel for one batch. loop_iters>0 wraps the body in a
    hardware loop (used by test.py for timing amplification only)."""
    bass, bacc, mybir, tile = _get_mods()
    FP8 = mybir.dt.float8e4
    BF16 = mybir.dt.bfloat16
    F32 = mybir.dt.float32

    class _Bacc(bacc.Bacc):
        def compile(self):
            super().compile()
            split_excess_waits(self, mybir)

    nc = _Bacc("TRN2", target_bir_lowering=False, debug=False)

    # DRAM I/O
    qt = nc.dram_tensor("qt", [512, 512], BF16, kind="ExternalInput")   # Q^T [c, i]
    kt = nc.dram_tensor("kt", [512, 512], BF16, kind="ExternalInput")   # K^T [c, j]
    vt = nc.dram_tensor("vt", [512, 512], BF16, kind="ExternalInput")   # V^T [c, j]
    wqt = nc.dram_tensor("wqt", [512, 512], BF16, kind="ExternalInput")  # Wq^T
    wkt = nc.dram_tensor("wkt", [512, 512], BF16, kind="ExternalInput")
    wvt = nc.dram_tensor("wvt", [512, 512], BF16, kind="ExternalInput")
    wot = nc.dram_tensor("wot", [8, 64, 512], BF16, kind="ExternalInput")  # Wo^T
    idn = nc.dram_tensor("idn", [128, 128], BF16, kind="ExternalInput")
    # ab2[hb, 64*half+d, ((p*4)+jb)*128+j'] = at[tb[32hb+2p+half, 128jb+j'], d]
    ab2 = nc.dram_tensor("ab2", [16, 128, 2, 4096], FP8, kind="ExternalInput")
    # mbd[jt, j', i] = -30*mask[i, 128jt+j']
    mbd = nc.dram_tensor("mbd", [4, 128, 512], BF16, kind="ExternalInput")
    # vtp[hb, jc, j', p*128 + 64*par + d] = vec_tab[tb[32hb+2p+par, 128jc+j'], d]
    vtp = nc.dram_tensor("vtp", [16, 4, 128, 16 * 128], FP8, kind="ExternalInput")
    out = nc.dram_tensor("out", [512, 512], F32, kind="ExternalOutput")

    with tile.TileContext(nc) as tc:
        with (
            tc.tile_pool(name="persist", bufs=1) as pp,
            tc.tile_pool(name="abstream", bufs=2) as absp,
            tc.tile_pool(name="vstream", bufs=4) as vsp,
            tc.tile_pool(name="stage", bufs=2) as stp,
            tc.tile_pool(name="psA", bufs=4, space="PSUM") as psA,
            tc.tile_pool(name="psB", bufs=3, space="PSUM") as psB,
        ):
            # ---- persistent tiles
            wq_s = pp.tile([128, 4 * 512], BF16, tag="wq")
            wk_s = pp.tile([128, 4 * 512], BF16, tag="wk")
            wv_s = pp.tile([128, 4 * 512], BF16, tag="wv")
            wo_s = pp.tile([64, 8 * 512], BF16, tag="wo")
            xbq = pp.tile([128, 4 * 512], BF16, tag="xbq")
            xbk = pp.tile([128, 4 * 512], BF16, tag="xbk")
            xbv = pp.tile([128, 4 * 512], BF16, tag="xbv")
            ident = pp.tile([128, 128], BF16, tag="ident")
            qDT = pp.tile([64, 8, 512], BF16, tag="qDT")     # [d, h, i]
            # q2[64*par+d, pg*16 + sd*8 + h] = q[h, 2pg+par, d]*c*(-1 if par&sd)
            q2 = pp.tile([128, 4096], BF16, tag="q2")
            kT8 = [pp.tile([64, 512], BF16, tag=f"kT{h}", name=f"kT{h}")
                   for h in range(8)]
            vaug = [pp.tile([128, 8, 65], BF16, tag=f"va{t}", name=f"va{t}")
                    for t in range(4)]
            s2sb = [pp.tile([128, 512, 8], BF16, tag=f"s2{t}", name=f"s2{t}")
                    for t in range(4)]                       # [j', i, h]
            mbt = [pp.tile([128, 512], BF16, tag=f"mb{t}", name=f"mb{t}")
                   for t in range(4)]
            attnU = [pp.tile([128, 8, 64, 8], BF16, tag=f"au{t}", name=f"au{t}")
                     for t in range(4)]                      # [j, h, t, g]
            o2sb = pp.tile([64, 512, 8], BF16, tag="o2sb")   # [d, i, h]
            oT8 = [pp.tile([64, 512], BF16, tag=f"oT{h}", name=f"oT{h}")
                   for h in range(8)]
            ones1 = pp.tile([1, 64], F32, tag="ones1")

            # loop-invariant constants
            nc.vector.memset(ones1[:], 1.0)
            for jt in range(4):
                nc.vector.memset(vaug[jt][:, :, 64], 1.0)

            def body():
                # ---- input DMAs on the Activation HWDGE queue, so the big
                # fp8 table streams own the SP queue from t=0
                for ck in range(4):
                    nc.scalar.dma_start(out=wq_s[:, ck * 512:(ck + 1) * 512],
                                        in_=wqt[ck * 128:(ck + 1) * 128, :])
                for ck in range(4):
                    nc.scalar.dma_start(out=xbq[:, ck * 512:(ck + 1) * 512],
                                        in_=qt[ck * 128:(ck + 1) * 128, :])
                nc.scalar.dma_start(out=ident[:], in_=idn[:, :])
                for ck in range(4):
                    nc.scalar.dma_start(out=wk_s[:, ck * 512:(ck + 1) * 512],
                                        in_=wkt[ck * 128:(ck + 1) * 128, :])
                    nc.scalar.dma_start(out=xbk[:, ck * 512:(ck + 1) * 512],
                                        in_=kt[ck * 128:(ck + 1) * 128, :])
                for ck in range(4):
                    nc.scalar.dma_start(out=wv_s[:, ck * 512:(ck + 1) * 512],
                                        in_=wvt[ck * 128:(ck + 1) * 128, :])
                    nc.scalar.dma_start(out=xbv[:, ck * 512:(ck + 1) * 512],
                                        in_=vt[ck * 128:(ck + 1) * 128, :])
                for jt in range(4):
                    nc.scalar.dma_start(out=mbt[jt][:], in_=mbd[jt, :, :])

                # ---- P1: projections (h-pair packed weight loads)
                scale = float(D ** -0.5)
                for hp in range(4):
                    pq = psA.tile([128, 512], F32, tag="psA")
                    for ck in range(4):
                        nc.tensor.matmul(
                            out=pq[:],
                            lhsT=wq_s[:, ck * 512 + hp * 128: ck * 512 + hp * 128 + 128],
                            rhs=xbq[:, ck * 512:(ck + 1) * 512],
                            start=(ck == 0), stop=(ck == 3))
                    for s in range(2):
                        h = 2 * hp + s
                        nc.scalar.activation(
                            out=qDT[:, h, :], in_=pq[s * 64:(s + 1) * 64, :],
                            func=mybir.ActivationFunctionType.Copy, scale=scale)
                        # q2 stacked pair columns (sum & diff), pre-scaled 0.5
                        q2v = q2[:].rearrange("p (pg sd h) -> p pg sd h",
                                              sd=2, h=8)
                        for par in range(2):
                            src = pq[s * 64:(s + 1) * 64, par::2]
                            for sd in range(2):
                                sgn = -1.0 if (par == 1 and sd == 1) else 1.0
                                nc.vector.tensor_scalar_mul(
                                    out=q2v[64 * par:64 * par + 64, :, sd, h],
                                    in0=src, scalar1=0.5 * scale * sgn)

                for hp in range(4):
                    pk = psA.tile([128, 512], F32, tag="psA")
                    for ck in range(4):
                        nc.tensor.matmul(
                            out=pk[:],
                            lhsT=wk_s[:, ck * 512 + hp * 128: ck * 512 + hp * 128 + 128],
                            rhs=xbk[:, ck * 512:(ck + 1) * 512],
                            start=(ck == 0), stop=(ck == 3))
                    for s in range(2):
                        h = 2 * hp + s
                        nc.scalar.activation(
                            out=kT8[h][:], in_=pk[s * 64:(s + 1) * 64, :],
                            func=mybir.ActivationFunctionType.Copy)

                for jt in range(4):  # v natural [j, hd]
                    pv = psA.tile([128, 512], F32, tag="psA")
                    for ck in range(4):
                        nc.tensor.matmul(
                            out=pv[:],
                            lhsT=xbv[:, ck * 512 + jt * 128: ck * 512 + (jt + 1) * 128],
                            rhs=wv_s[:, ck * 512:(ck + 1) * 512],
                            start=(ck == 0), stop=(ck == 3))
                    for h in range(8):
                        nc.vector.tensor_copy(out=vaug[jt][:, h, 0:64],
                                              in_=pv[:, h * 64:(h + 1) * 64])

                # ---- P2: score2 via K-packed sum/diff pair matmuls
                for hb in range(16):
                    abt = absp.tile([128, 8192], FP8, tag="ab2")
                    nc.sync.dma_start(out=abt[:], in_=ab2[hb, :, :, :])
                    for jb in range(4):
                        ps2 = psA.tile([128, 256], F32, tag="psA")
                        for p in range(16):
                            pg = 16 * hb + p
                            nc.tensor.matmul(
                                out=ps2[:, p * 16:(p + 1) * 16],
                                lhsT=abt[:, (p * 4 + jb) * 128:(p * 4 + jb + 1) * 128],
                                rhs=q2[:, pg * 16:(pg + 1) * 16],
                                start=True, stop=True)
                        # recombine: s0 = sum+diff, s1 = sum-diff
                        # (DVE reads at most one PSUM operand: stage diff)
                        pv2 = ps2[:].rearrange("p (q sd h) -> p q sd h",
                                               sd=2, h=8)
                        dcp = stp.tile([128, 16, 8], BF16, tag="dcp")
                        nc.vector.tensor_copy(out=dcp[:, :, :],
                                              in_=pv2[:, :, 1, :])
                        dst = s2sb[jb][:, 32 * hb:32 * hb + 32, :].rearrange(
                            "p (q two) h -> p q two h", two=2)
                        nc.vector.tensor_add(out=dst[:, :, 0, :],
                                             in0=pv2[:, :, 0, :],
                                             in1=dcp[:, :, :])
                        nc.vector.tensor_sub(out=dst[:, :, 1, :],
                                             in0=pv2[:, :, 0, :],
                                             in1=dcp[:, :, :])

                # ---- P3: qk + s2 + mask (psum accumulate) -> exp -> attnU
                for jt in range(4):
                    for h in range(8):
                        pS = psB.tile([128, 512], F32, tag="psB")
                        nc.tensor.matmul(
                            out=pS[:],
                            lhsT=kT8[h][:, jt * 128:(jt + 1) * 128],
                            rhs=qDT[:, h, :],
                            start=True, stop=False)
                        nc.tensor.matmul(
                            out=pS[:], lhsT=ident[:], rhs=s2sb[jt][:, :, h],
                            start=False, stop=False)
                        nc.tensor.matmul(
                            out=pS[:], lhsT=ident[:], rhs=mbt[jt][:],
                            start=False, stop=True)
                        nc.scalar.activation(out=attnU[jt][:, h, :, :], in_=pS[:],
                                             func=mybir.ActivationFunctionType.Exp)

                # ---- P4: bias-AV via M-packed pair fp8 matmuls
                for hb in range(16):
                    vtb = vsp.tile([128, 4, 16 * 128], FP8, tag="vtb")
                    for jc in range(4):
                        nc.sync.dma_start(out=vtb[:, jc, :],
                                          in_=vtp[hb, jc, :, :])
                    po2 = psA.tile([128, 256], F32, tag="psA")
                    for p in range(16):
                        i0 = 32 * hb + 2 * p
                        tt, gg = i0 // 8, i0 % 8
                        for jc in range(4):
                            nc.tensor.matmul(
                                out=po2[:, p * 16:(p + 1) * 16],
                                lhsT=vtb[:, jc, p * 128:(p + 1) * 128],
                                rhs=attnU[jc][:, :, tt, gg:gg + 2],
                                start=(jc == 0), stop=(jc == 3))
                    src_e = po2[0:64].rearrange("p (q n) -> p q n", n=16)
                    src_o = po2[64:128].rearrange("p (q n) -> p q n", n=16)
                    dst = o2sb[:, 32 * hb:32 * hb + 32, :].rearrange(
                        "p (q two) h -> p q two h", two=2)
                    nc.vector.tensor_copy(
                        out=dst[:, :, 0, :],
                        in_=src_e.rearrange("p q (h two) -> p q two h",
                                            two=2)[:, :, 0, :])
                    nc.vector.tensor_copy(
                        out=dst[:, :, 1, :],
                        in_=src_o.rearrange("p q (h two) -> p q two h",
                                            two=2)[:, :, 1, :])

                # ---- P5: AV (+Z) and combine
                for hc in range(8):
                    nc.sync.dma_start(out=wo_s[:, hc * 512:(hc + 1) * 512],
                                      in_=wot[hc, :, :])
                for h in range(8):
                    po1 = psB.tile([65, 512], F32, tag="psB")
                    for jc in range(4):
                        nc.tensor.matmul(
                            out=po1[:],
                            lhsT=vaug[jc][:, h, :],
                            rhs=attnU[jc][:, h, :, :],
                            start=(jc == 0), stop=(jc == 3))
                    rz = stp.tile([1, 512], F32, tag="rz")
                    nc.vector.reciprocal(out=rz[:], in_=po1[64:65, :])
                    rzP = psA.tile([64, 512], F32, tag="psA")
                    nc.tensor.matmul(out=rzP[:], lhsT=ones1[:], rhs=rz[:],
                                     start=True, stop=True)
                    tmp = stp.tile([64, 512], F32, tag="cmb")
                    nc.vector.tensor_add(out=tmp[:], in0=po1[0:64, :],
                                         in1=o2sb[:, :, h])
                    nc.vector.tensor_mul(out=oT8[h][:], in0=tmp[:], in1=rzP[:])

                # ---- P6: output projection
                for it in range(4):
                    po = psA.tile([128, 512], F32, tag="psA")
                    for hc in range(8):
                        nc.tensor.matmul(out=po[:],
                                         lhsT=oT8[hc][:, it * 128:(it + 1) * 128],
                                         rhs=wo_s[:, hc * 512:(hc + 1) * 512],
                                         start=(hc == 0), stop=(hc == 7))
                    od = stp.tile([128, 512], F32, tag="od")
                    nc.vector.tensor_copy(out=od[:], in_=po[:])
                    nc.sync.dma_start(out=out[it * 128:(it + 1) * 128, :], in_=od[:])

            if loop_iters > 0:
                hint = (mybir.EngineType.PE, mybir.EngineType.DVE,
                        mybir.EngineType.Activation, mybir.EngineType.SP)
                with tc.For_i(0, loop_iters, 1, hint_engines=hint) as _:
                    body()
            else:
                body()

    nc.finalize()
    return nc


# ---------------------------------------------------------------- host side
def _host_prep(inputs):
    import ml_dtypes
    import concourse.mybir as mybir
    FP8NP = mybir.dt.np(mybir.dt.float8e4)
    BF16NP = ml_dtypes.bfloat16

    Q = np.asarray(inputs["Q"], np.float32)
    K = np.asarray(inputs["K"], np.float32)
    V = np.asarray(inputs["V"], np.float32)
    mask = np.asarray(inputs["mask"], bool)
    tb = np.asarray(inputs["time_bias"], np.int64)
    Wq = np.asarray(inputs["Wq"], np.float32)
    Wk = np.asarray(inputs["Wk"], np.float32)
    Wv = np.asarray(inputs["Wv"], np.float32)
    Wo = np.asarray(inputs["Wo"], np.float32)
    at = np.asarray(inputs["att_bias_tab"], np.float32)
    vt_tab = np.asarray(inputs["vec_bias_tab"], np.float32)

    wqt = np.ascontiguousarray(Wq.T).astype(BF16NP)
    wkt = np.ascontiguousarray(Wk.T).astype(BF16NP)
    wvt = np.ascontiguousarray(Wv.T).astype(BF16NP)
    wot = np.ascontiguousarray(Wo.T).astype(BF16NP).reshape(8, 64, 512)
    ident = np.eye(128, dtype=np.float32).astype(BF16NP)

    at8 = at.astype(FP8NP)                                   # [183, 64]
    vt8 = vt_tab.astype(FP8NP)

    in_maps = []
    for b in range(B):
        # ab2: [hb, (half,d), p, jb, j']
        Ga = at8[tb[b]]                                      # [512 i, 512 j, 64]
        arr = Ga.reshape(16, 16, 2, 4, 128, 64)              # hb p half jb j' d
        arr = arr.transpose(0, 2, 5, 1, 3, 4)                # hb half d p jb j'
        ab2 = np.ascontiguousarray(arr.reshape(16, 128, 2, 4096))
        # mbias
        mbd = np.where(mask[b].T, np.float32(MASK_NEG),
                       np.float32(0.0)).astype(BF16NP).reshape(4, 128, 512)
        # vtp pair-packed
        Gv = vt8[tb[b]]                                      # [512 i, 512 j, 64]
        Gv = np.ascontiguousarray(Gv.transpose(1, 0, 2))     # [j, i, d]
        vtp = Gv.reshape(4, 128, 16, 32, 64).transpose(2, 0, 1, 3, 4)
        vtp = np.ascontiguousarray(vtp.reshape(16, 4, 128, 16 * 128))
        in_maps.append({
            "qt": np.ascontiguousarray(Q[b].T).astype(BF16NP),
            "kt": np.ascontiguousarray(K[b].T).astype(BF16NP),
            "vt": np.ascontiguousarray(V[b].T).astype(BF16NP),
            "wqt": wqt, "wkt": wkt, "wvt": wvt, "wot": wot,
            "idn": ident, "ab2": ab2, "mbd": mbd,
            "vtp": vtp,
        })
    return in_maps


def kernel(**inputs):
    from concourse.bass_utils import run_bass_kernel_spmd
    key = "main"
    if key not in _NC_CACHE:
        _NC_CACHE[key] = build_kernel()
    nc = _NC_CACHE[key]
    in_maps = _host_prep(inputs)
    res = run_bass_kernel_spmd(nc, in_maps, core_ids=list(range(8)), trace=False)
    out = np.stack([res.results[b]["out"] for b in range(B)], axis=0)
    return out.astype(np.float32)


# revision 23
# speedup vs baseline: 1.1104x; 1.1104x over previous
"""nn_MultiHeadAttention sparse-attention kernel for 8 TRN2 NeuronCores.

Strategy: batch-parallel (B=8 -> 1 batch per core). Per-(i,j) bias terms:

  score2[h,i,j] = q[h,i,:].att_tab[tb[i,j]]        (+ -30*mask)
  out2[h,i,d]   = sum_j attn[h,i,j]*vec_tab[tb[i,j],d]

Both use host-expanded fp8 pair-tensors so TWO i values share each 128-column
PE weight load (weight loads, not FLOPs, bound these phases on HW):
 - score2: K-packed sum/diff trick. lhsT rows = [at[tb[i0,j]] | at[tb[i1,j]]],
   rhs cols = [q_i0;q_i1]/2 and [q_i0;-q_i1]/2 -> psum holds (s0+s1)/2 and
   (s0-s1)/2; a DVE add/sub pair recovers s0, s1. Mask is applied later as a
   bf16 identity-matmul accumulate (mbias) into the qk psum.
 - out2: M-packed: lhsT cols = [vt[tb[i0,j]] | vt[tb[i1,j]]], rhs = attn
   column pairs (N=16); valid halves are scattered out by two DVE copies.
Softmax denominators come free from a ones-column in the v weight tiles.
"""
import sys
import numpy as np

sys.path.insert(0, "/opt/trn_rl_repo")

HEADS = 8
B, L, HID = 8, 512, 512
D = HID // HEADS
NB = 183
MASK_NEG = -30.0

_NC_CACHE = {}


# ---------------------------------------------------------------- bass build
def _get_mods():
    import concourse.bass as bass
    import concourse.bacc as bacc
    import concourse.mybir as mybir
    import concourse.tile as tile
    return bass, bacc, mybir, tile


def split_excess_waits(nc, mybir):
    """This container's walrus supports only 1 sync wait on TPB_CTRL
    instructions (Drain/NoOp); split extras onto preceding 1-wait NoOps."""
    limited = ("Drain", "NoOp", "AllEngineBarrier", "Halt")
    for f in nc.m.functions:
        for bb in f.blocks:
            new_insts = []
            for inst in bb.instructions:
                si = inst.sync_info
                if (inst.opcode in limited and si is not None and si.on_wait
                        and len(si.on_wait) > 1):
                    waits = list(si.on_wait)
                    keep, extra = waits[:1], waits[1:]
                    eng = nc.engines[inst.engine]
                    for w in extra:
                        nop = eng.nop(hint="waitsplit", nofuse=True)
                        nopinst = nop.ins
                        for fb in nc.m.functions:
                            for bb2 in fb.blocks:
                                if nopinst in bb2.instructions:
                                    bb2.instructions.remove(nopinst)
                        nopinst.sync_info = mybir.SyncInfo(on_wait=[w], on_update=[])
                        new_insts.append(nopinst)
                    si.on_wait = keep
                new_insts.append(inst)
            bb.instructions[:] = new_insts


def build_kernel(loop_iters=0):
    """One-core kernel for one batch. loop_iters>0 wraps the body in a
    hardware loop (used by test.py for timing amplification only)."""
    bass, bacc, mybir, tile = _get_mods()
    FP8 = mybir.dt.float8e4
    BF16 = mybir.dt.bfloat16
    F32 = mybir.dt.float32

    class _Bacc(bacc.Bacc):
        def compile(self):
            super().compile()
            split_excess_waits(self, mybir)

    nc = _Bacc("TRN2", target_bir_lowering=False, debug=False)

    # DRAM I/O
    qt = nc.dram_tensor("qt", [512, 512], BF16, kind="ExternalInput")   # Q^T [c, i]
    kt = nc.dram_tensor("kt", [512, 512], BF16, kind="ExternalInput")   # K^T [c, j]
    vt = nc.dram_tensor("vt", [512, 512], BF16, kind="ExternalInput")   # V^T [c, j]
    wqt = nc.dram_tensor("wqt", [512, 512], BF16, kind="ExternalInput")  # Wq^T
    wkt = nc.dram_tensor("wkt", [512, 512], BF16, kind="ExternalInput")
    wvt = nc.dram_tensor("wvt", [512, 512], BF16, kind="ExternalInput")
    wot = nc.dram_tensor("wot", [8, 64, 512], BF16, kind="ExternalInput")  # Wo^T
    idn = nc.dram_tensor("idn", [128, 128], BF16, kind="ExternalInput")
    # ab2[hb, 64*half+d, ((p*4)+jb)*128+j'] = at[tb[32hb+2p+half, 128jb+j'], d]
    ab2 = nc.dram_tensor("ab2", [16, 128, 2, 4096], FP8, kind="ExternalInput")
    # mbd[jt, j', i] = -30*mask[i, 128jt+j']
    mbd = nc.dram_tensor("mbd", [4, 128, 512], BF16, kind="ExternalInput")
    # vtp[hb, jc, j', p*128 + 64*par + d] = vec_tab[tb[32hb+2p+par, 128jc+j'], d]
    vtp = nc.dram_tensor("vtp", [16, 4, 128, 16 * 128], FP8, kind="ExternalInput")
    out = nc.dram_tensor("out", [512, 512], F32, kind="ExternalOutput")

    with tile.TileContext(nc) as tc:
        with (
            tc.tile_pool(name="persist", bufs=1) as pp,
            tc.tile_pool(name="tabstream", bufs=6) as tsp,
            tc.tile_pool(name="stage", bufs=2) as stp,
            tc.tile_pool(name="psA", bufs=2, space="PSUM") as psA,
            tc.tile_pool(name="psB", bufs=2, space="PSUM") as psB,
            tc.tile_pool(name="psC", bufs=2, space="PSUM") as psC,
        ):
            # ---- persistent tiles
            wq_s = pp.tile([128, 4 * 512], BF16, tag="wq")
            wk_s = pp.tile([128, 4 * 512], BF16, tag="wk")
            wv_s = pp.tile([128, 4 * 512], BF16, tag="wv")
            wo_s = pp.tile([64, 8 * 512], BF16, tag="wo")
            xbq = pp.tile([128, 4 * 512], BF16, tag="xbq")
            xbk = pp.tile([128, 4 * 512], BF16, tag="xbk")
            xbv = pp.tile([128, 4 * 512], BF16, tag="xbv")
            ident = pp.tile([128, 128], BF16, tag="ident")
            qDT = pp.tile([64, 8, 512], BF16, tag="qDT")     # [d, h, i]
            # q2[64*par+d, pg*16 + sd*8 + h] = q[h, 2pg+par, d]*c*(-1 if par&sd)
            q2 = pp.tile([128, 4096], BF16, tag="q2")
            kT8 = [pp.tile([64, 512], BF16, tag=f"kT{h}", name=f"kT{h}")
                   for h in range(8)]
            vaug = [pp.tile([128, 8, 65], BF16, tag=f"va{t}", name=f"va{t}")
                    for t in range(4)]
            s2one = pp.tile([128, 4, 512, 8], BF16, tag="s2one")  # [j', jt, i, h]
            mbt = [pp.tile([128, 512], BF16, tag=f"mb{t}", name=f"mb{t}")
                   for t in range(4)]
            attnU = [pp.tile([128, 8, 64, 8], BF16, tag=f"au{t}", name=f"au{t}")
                     for t in range(4)]                      # [j, h, t, g]
            o2sb = pp.tile([64, 512, 8], BF16, tag="o2sb")   # [d, i, h]
            oT8 = [pp.tile([64, 512], BF16, tag=f"oT{h}", name=f"oT{h}")
                   for h in range(8)]
            ones1 = pp.tile([1, 64], F32, tag="ones1")

            # loop-invariant constants
            nc.vector.memset(ones1[:], 1.0)
            for jt in range(4):
                nc.vector.memset(vaug[jt][:, :, 64], 1.0)

            def body():
                # ---- input DMAs
                for ck in range(4):
                    nc.sync.dma_start(out=wq_s[:, ck * 512:(ck + 1) * 512],
                                      in_=wqt[ck * 128:(ck + 1) * 128, :])
                for ck in range(4):
                    nc.sync.dma_start(out=xbq[:, ck * 512:(ck + 1) * 512],
                                      in_=qt[ck * 128:(ck + 1) * 128, :])
                nc.sync.dma_start(out=ident[:], in_=idn[:, :])
                for ck in range(4):
                    nc.sync.dma_start(out=wk_s[:, ck * 512:(ck + 1) * 512],
                                      in_=wkt[ck * 128:(ck + 1) * 128, :])
                    nc.sync.dma_start(out=xbk[:, ck * 512:(ck + 1) * 512],
                                      in_=kt[ck * 128:(ck + 1) * 128, :])
                for ck in range(4):
                    nc.sync.dma_start(out=wv_s[:, ck * 512:(ck + 1) * 512],
                                      in_=wvt[ck * 128:(ck + 1) * 128, :])
                    nc.sync.dma_start(out=xbv[:, ck * 512:(ck + 1) * 512],
                                      in_=vt[ck * 128:(ck + 1) * 128, :])
                for jt in range(4):
                    nc.sync.dma_start(out=mbt[jt][:], in_=mbd[jt, :, :])

                # ---- P1: projections (h-pair packed weight loads)
                scale = float(D ** -0.5)
                for hp in range(4):
                    pq = psA.tile([128, 512], F32, tag="psA")
                    for ck in range(4):
                        nc.tensor.matmul(
                            out=pq[:],
                            lhsT=wq_s[:, ck * 512 + hp * 128: ck * 512 + hp * 128 + 128],
                            rhs=xbq[:, ck * 512:(ck + 1) * 512],
                            start=(ck == 0), stop=(ck == 3))
                    for s in range(2):
                        h = 2 * hp + s
                        nc.scalar.activation(
                            out=qDT[:, h, :], in_=pq[s * 64:(s + 1) * 64, :],
                            func=mybir.ActivationFunctionType.Copy, scale=scale)
                        # q2 stacked pair columns (sum & diff), pre-scaled 0.5
                        q2v = q2[:].rearrange("p (pg sd h) -> p pg sd h",
                                              sd=2, h=8)
                        for par in range(2):
                            src = pq[s * 64:(s + 1) * 64, par::2]
                            for sd in range(2):
                                sgn = -1.0 if (par == 1 and sd == 1) else 1.0
                                nc.vector.tensor_scalar_mul(
                                    out=q2v[64 * par:64 * par + 64, :, sd, h],
                                    in0=src, scalar1=0.5 * scale * sgn)

                for hp in range(4):
                    pk = psA.tile([128, 512], F32, tag="psA")
                    for ck in range(4):
                        nc.tensor.matmul(
                            out=pk[:],
                            lhsT=wk_s[:, ck * 512 + hp * 128: ck * 512 + hp * 128 + 128],
                            rhs=xbk[:, ck * 512:(ck + 1) * 512],
                            start=(ck == 0), stop=(ck == 3))
                    for s in range(2):
                        h = 2 * hp + s
                        nc.scalar.activation(
                            out=kT8[h][:], in_=pk[s * 64:(s + 1) * 64, :],
                            func=mybir.ActivationFunctionType.Copy)

                for jt in range(4):  # v natural [j, hd]
                    pv = psA.tile([128, 512], F32, tag="psA")
                    for ck in range(4):
                        nc.tensor.matmul(
                            out=pv[:],
                            lhsT=xbv[:, ck * 512 + jt * 128: ck * 512 + (jt + 1) * 128],
                            rhs=wv_s[:, ck * 512:(ck + 1) * 512],
                            start=(ck == 0), stop=(ck == 3))
                    for h in range(8):
                        nc.vector.tensor_copy(out=vaug[jt][:, h, 0:64],
                                              in_=pv[:, h * 64:(h + 1) * 64])

                # ---- P2: score2 via K-packed sum/diff pair matmuls
                for hb in range(16):
                    abt = tsp.tile([128, 8192], FP8, tag="strm")
                    nc.sync.dma_start(out=abt[:], in_=ab2[hb, :, :, :])
                    psc = psC.tile([128, 4, 16, 2, 8], F32, tag="psC")
                    for jb in range(4):
                        for p in range(16):
                            pg = 16 * hb + p
                            nc.tensor.matmul(
                                out=psc[:, jb, p, :, :],
                                lhsT=abt[:, (p * 4 + jb) * 128:(p * 4 + jb + 1) * 128],
                                rhs=q2[:, pg * 16:(pg + 1) * 16],
                                start=True, stop=True)
                    # recombine all 4 jb at once: s0 = sum+diff, s1 = sum-diff
                    # (DVE reads at most one PSUM operand: stage diff)
                    dcp = stp.tile([128, 4, 16, 8], BF16, tag="dcp")
                    nc.vector.tensor_copy(out=dcp[:, :, :, :],
                                          in_=psc[:, :, :, 1, :])
                    dst = s2one[:, :, 32 * hb:32 * hb + 32, :].rearrange(
                        "p jb (q two) h -> p jb q two h", two=2)
                    nc.vector.tensor_add(out=dst[:, :, :, 0, :],
                                         in0=psc[:, :, :, 0, :],
                                         in1=dcp[:, :, :, :])
                    nc.vector.tensor_sub(out=dst[:, :, :, 1, :],
                                         in0=psc[:, :, :, 0, :],
                                         in1=dcp[:, :, :, :])

                # ---- P3: qk + s2 + mask (psum accumulate) -> exp -> attnU
                for jt in range(4):
                    for h in range(8):
                        pS = psB.tile([128, 512], F32, tag="psB")
                        nc.tensor.matmul(
                            out=pS[:],
                            lhsT=kT8[h][:, jt * 128:(jt + 1) * 128],
                            rhs=qDT[:, h, :],
                            start=True, stop=False)
                        nc.tensor.matmul(
                            out=pS[:], lhsT=ident[:], rhs=s2one[:, jt, :, h],
                            start=False, stop=False)
                        nc.tensor.matmul(
                            out=pS[:], lhsT=ident[:], rhs=mbt[jt][:],
                            start=False, stop=True)
                        nc.scalar.activation(out=attnU[jt][:, h, :, :], in_=pS[:],
                                             func=mybir.ActivationFunctionType.Exp)

                # ---- P4: bias-AV via M-packed pair fp8 matmuls
                for hb in range(16):
                    vtb = tsp.tile([128, 4, 16 * 128], FP8, tag="strm")
                    for jc in range(4):
                        nc.sync.dma_start(out=vtb[:, jc, :],
                                          in_=vtp[hb, jc, :, :])
                    po2 = psA.tile([128, 256], F32, tag="psA")
                    for p in range(16):
                        i0 = 32 * hb + 2 * p
                        tt, gg = i0 // 8, i0 % 8
                        for jc in range(4):
                            nc.tensor.matmul(
                                out=po2[:, p * 16:(p + 1) * 16],
                                lhsT=vtb[:, jc, p * 128:(p + 1) * 128],
                                rhs=attnU[jc][:, :, tt, gg:gg + 2],
                                start=(jc == 0), stop=(jc == 3))
                    src_e = po2[0:64].rearrange("p (q n) -> p q n", n=16)
                    src_o = po2[64:128].rearrange("p (q n) -> p q n", n=16)
                    dst = o2sb[:, 32 * hb:32 * hb + 32, :].rearrange(
                        "p (q two) h -> p q two h", two=2)
                    nc.vector.tensor_copy(
                        out=dst[:, :, 0, :],
                        in_=src_e.rearrange("p q (h two) -> p q two h",
                                            two=2)[:, :, 0, :])
                    nc.vector.tensor_copy(
                        out=dst[:, :, 1, :],
                        in_=src_o.rearrange("p q (h two) -> p q two h",
                                            two=2)[:, :, 1, :])

                # ---- P5: AV (+Z) and combine
                for hc in range(8):
                    nc.sync.dma_start(out=wo_s[:, hc * 512:(hc + 1) * 512],
                                      in_=wot[hc, :, :])
                for h in range(8):
                    po1 = psB.tile([65, 512], F32, tag="psB")
                    for jc in range(4):
                        nc.tensor.matmul(
                            out=po1[:],
                            lhsT=vaug[jc][:, h, :],
                            rhs=attnU[jc][:, h, :, :],
                            start=(jc == 0), stop=(jc == 3))
                    rz = stp.tile([1, 512], F32, tag="rz")
                    nc.vector.reciprocal(out=rz[:], in_=po1[64:65, :])
                    rzP = psA.tile([64, 512], F32, tag="psA")
                    nc.tensor.matmul(out=rzP[:], lhsT=ones1[:], rhs=rz[:],
                                     start=True, stop=True)
                    tmp = stp.tile([64, 512], F32, tag="cmb")
                    nc.vector.tensor_add(out=tmp[:], in0=po1[0:64, :],
                                         in1=o2sb[:, :, h])
                    nc.vector.tensor_mul(out=oT8[h][:], in0=tmp[:], in1=rzP[:])

                # ---- P6: output projection
                for it in range(4):
                    po = psA.tile([128, 512], F32, tag="psA")
                    for hc in range(8):
                        nc.tensor.matmul(out=po[:],
                                         lhsT=oT8[hc][:, it * 128:(it + 1) * 128],
                                         rhs=wo_s[:, hc * 512:(hc + 1) * 512],
                                         start=(hc == 0), stop=(hc == 7))
                    od = stp.tile([128, 512], F32, tag="od")
                    nc.vector.tensor_copy(out=od[:], in_=po[:])
                    nc.sync.dma_start(out=out[it * 128:(it + 1) * 128, :], in_=od[:])

            if loop_iters > 0:
                hint = (mybir.EngineType.PE, mybir.EngineType.DVE,
                        mybir.EngineType.Activation, mybir.EngineType.SP)
                with tc.For_i(0, loop_iters, 1, hint_engines=hint) as _:
                    body()
            else:
                body()

    nc.finalize()
    return nc


# ---------------------------------------------------------------- host side
def _host_prep(inputs):
    import ml_dtypes
    import concourse.mybir as mybir
    FP8NP = mybir.dt.np(mybir.dt.float8e4)
    BF16NP = ml_dtypes.bfloat16

    Q = np.asarray(inputs["Q"], np.float32)
    K = np.asarray(inputs["K"], np.float32)
    V = np.asarray(inputs["V"], np.float32)
    mask = np.asarray(inputs["mask"], bool)
    tb = np.asarray(inputs["time_bias"], np.int64)
    Wq = np.asarray(inputs["Wq"], np.float32)
    Wk = np.asarray(inputs["Wk"], np.float32)
    Wv = np.asarray(inputs["Wv"], np.float32)
    Wo = np.asarray(inputs["Wo"], np.float32)
    at = np.asarray(inputs["att_bias_tab"], np.float32)
    vt_tab = np.asarray(inputs["vec_bias_tab"], np.float32)

    wqt = np.ascontiguousarray(Wq.T).astype(BF16NP)
    wkt = np.ascontiguousarray(Wk.T).astype(BF16NP)
    wvt = np.ascontiguousarray(Wv.T).astype(BF16NP)
    wot = np.ascontiguousarray(Wo.T).astype(BF16NP).reshape(8, 64, 512)
    ident = np.eye(128, dtype=np.float32).astype(BF16NP)

    at8 = at.astype(FP8NP)                                   # [183, 64]
    vt8 = vt_tab.astype(FP8NP)

    in_maps = []
    for b in range(B):
        # ab2: [hb, (half,d), p, jb, j']
        Ga = at8[tb[b]]                                      # [512 i, 512 j, 64]
        arr = Ga.reshape(16, 16, 2, 4, 128, 64)              # hb p half jb j' d
        arr = arr.transpose(0, 2, 5, 1, 3, 4)                # hb half d p jb j'
        ab2 = np.ascontiguousarray(arr.reshape(16, 128, 2, 4096))
        # mbias
        mbd = np.where(mask[b].T, np.float32(MASK_NEG),
                       np.float32(0.0)).astype(BF16NP).reshape(4, 128, 512)
        # vtp pair-packed
        Gv = vt8[tb[b]]                                      # [512 i, 512 j, 64]
        Gv = np.ascontiguousarray(Gv.transpose(1, 0, 2))     # [j, i, d]
        vtp = Gv.reshape(4, 128, 16, 32, 64).transpose(2, 0, 1, 3, 4)
        vtp = np.ascontiguousarray(vtp.reshape(16, 4, 128, 16 * 128))
        in_maps.append({
            "qt": np.ascontiguousarray(Q[b].T).astype(BF16NP),
            "kt": np.ascontiguousarray(K[b].T).astype(BF16NP),
            "vt": np.ascontiguousarray(V[b].T).astype(BF16NP),
            "wqt": wqt, "wkt": wkt, "wvt": wvt, "wot": wot,
            "idn": ident, "ab2": ab2, "mbd": mbd,
            "vtp": vtp,
        })
    return in_maps


def kernel(**inputs):
    from concourse.bass_utils import run_bass_kernel_spmd
    key = "main"
    if key not in _NC_CACHE:
        _NC_CACHE[key] = build_kernel()
    nc = _NC_CACHE[key]
    in_maps = _host_prep(inputs)
    res = run_bass_kernel_spmd(nc, in_maps, core_ids=list(range(8)), trace=False)
    out = np.stack([res.results[b]["out"] for b in range(B)], axis=0)
    return out.astype(np.float32)


# revision 24
# speedup vs baseline: 1.1435x; 1.0298x over previous
"""nn_MultiHeadAttention sparse-attention kernel for 8 TRN2 NeuronCores.

Strategy: batch-parallel (B=8 -> 1 batch per core). Per-(i,j) bias terms:

  score2[h,i,j] = q[h,i,:].att_tab[tb[i,j]]        (+ -30*mask)
  out2[h,i,d]   = sum_j attn[h,i,j]*vec_tab[tb[i,j],d]

Both use host-expanded fp8 pair-tensors so TWO i values share each 128-column
PE weight load (weight loads, not FLOPs, bound these phases on HW):
 - score2: K-packed sum/diff trick. lhsT rows = [at[tb[i0,j]] | at[tb[i1,j]]],
   rhs cols = [q_i0;q_i1]/2 and [q_i0;-q_i1]/2 -> psum holds (s0+s1)/2 and
   (s0-s1)/2; a DVE add/sub pair recovers s0, s1. Mask is applied later as a
   bf16 identity-matmul accumulate (mbias) into the qk psum.
 - out2: M-packed: lhsT cols = [vt[tb[i0,j]] | vt[tb[i1,j]]], rhs = attn
   column pairs (N=16); valid halves are scattered out by two DVE copies.
Softmax denominators come free from a ones-column in the v weight tiles.
"""
import sys
import numpy as np

sys.path.insert(0, "/opt/trn_rl_repo")

HEADS = 8
B, L, HID = 8, 512, 512
D = HID // HEADS
NB = 183
MASK_NEG = -30.0

_NC_CACHE = {}


# ---------------------------------------------------------------- bass build
def _get_mods():
    import concourse.bass as bass
    import concourse.bacc as bacc
    import concourse.mybir as mybir
    import concourse.tile as tile
    return bass, bacc, mybir, tile


def split_excess_waits(nc, mybir):
    """This container's walrus supports only 1 sync wait on TPB_CTRL
    instructions (Drain/NoOp); split extras onto preceding 1-wait NoOps."""
    limited = ("Drain", "NoOp", "AllEngineBarrier", "Halt")
    for f in nc.m.functions:
        for bb in f.blocks:
            new_insts = []
            for inst in bb.instructions:
                si = inst.sync_info
                if (inst.opcode in limited and si is not None and si.on_wait
                        and len(si.on_wait) > 1):
                    waits = list(si.on_wait)
                    keep, extra = waits[:1], waits[1:]
                    eng = nc.engines[inst.engine]
                    for w in extra:
                        nop = eng.nop(hint="waitsplit", nofuse=True)
                        nopinst = nop.ins
                        for fb in nc.m.functions:
                            for bb2 in fb.blocks:
                                if nopinst in bb2.instructions:
                                    bb2.instructions.remove(nopinst)
                        nopinst.sync_info = mybir.SyncInfo(on_wait=[w], on_update=[])
                        new_insts.append(nopinst)
                    si.on_wait = keep
                new_insts.append(inst)
            bb.instructions[:] = new_insts


def build_kernel(loop_iters=0):
    """One-core kernel for one batch. loop_iters>0 wraps the body in a
    hardware loop (used by test.py for timing amplification only)."""
    bass, bacc, mybir, tile = _get_mods()
    FP8 = mybir.dt.float8e4
    BF16 = mybir.dt.bfloat16
    F32 = mybir.dt.float32

    class _Bacc(bacc.Bacc):
        def compile(self):
            super().compile()
            split_excess_waits(self, mybir)

    nc = _Bacc("TRN2", target_bir_lowering=False, debug=False)

    # DRAM I/O
    qt = nc.dram_tensor("qt", [512, 512], BF16, kind="ExternalInput")   # Q^T [c, i]
    kt = nc.dram_tensor("kt", [512, 512], BF16, kind="ExternalInput")   # K^T [c, j]
    vt = nc.dram_tensor("vt", [512, 512], BF16, kind="ExternalInput")   # V^T [c, j]
    wqt = nc.dram_tensor("wqt", [512, 512], BF16, kind="ExternalInput")  # Wq^T
    wkt = nc.dram_tensor("wkt", [512, 512], BF16, kind="ExternalInput")
    wvt = nc.dram_tensor("wvt", [512, 512], BF16, kind="ExternalInput")
    wot = nc.dram_tensor("wot", [8, 64, 512], BF16, kind="ExternalInput")  # Wo^T
    idn = nc.dram_tensor("idn", [128, 128], BF16, kind="ExternalInput")
    # ab2[hb, 64*half+d, ((p*4)+jb)*128+j'] = at[tb[32hb+2p+half, 128jb+j'], d]
    ab2 = nc.dram_tensor("ab2", [16, 128, 2, 4096], FP8, kind="ExternalInput")
    # mbd[jt, j', i] = -30*mask[i, 128jt+j']
    mbd = nc.dram_tensor("mbd", [4, 128, 512], BF16, kind="ExternalInput")
    # vtp[hb, jc, j', p*128 + 64*par + d] = vec_tab[tb[32hb+2p+par, 128jc+j'], d]
    vtp = nc.dram_tensor("vtp", [16, 4, 128, 16 * 128], FP8, kind="ExternalInput")
    out = nc.dram_tensor("out", [512, 512], F32, kind="ExternalOutput")

    with tile.TileContext(nc) as tc:
        with (
            tc.tile_pool(name="persist", bufs=1) as pp,
            tc.tile_pool(name="tabstream", bufs=6) as tsp,
            tc.tile_pool(name="stage", bufs=2) as stp,
            tc.tile_pool(name="psA", bufs=2, space="PSUM") as psA,
            tc.tile_pool(name="psB", bufs=2, space="PSUM") as psB,
            tc.tile_pool(name="psC", bufs=2, space="PSUM") as psC,
        ):
            # ---- persistent tiles
            wq_s = pp.tile([128, 4 * 512], BF16, tag="wq")
            wk_s = pp.tile([128, 4 * 512], BF16, tag="wk")
            wv_s = pp.tile([128, 4 * 512], BF16, tag="wv")
            wo_s = pp.tile([64, 8 * 512], BF16, tag="wo")
            xbq = pp.tile([128, 4 * 512], BF16, tag="xbq")
            xbk = pp.tile([128, 4 * 512], BF16, tag="xbk")
            xbv = pp.tile([128, 4 * 512], BF16, tag="xbv")
            ident = pp.tile([128, 128], BF16, tag="ident")
            qDT = pp.tile([64, 8, 512], BF16, tag="qDT")     # [d, h, i]
            # q2[64*par+d, pg*16 + sd*8 + h] = q[h, 2pg+par, d]*c*(-1 if par&sd)
            q2 = pp.tile([128, 4096], BF16, tag="q2")
            kT8 = [pp.tile([64, 512], BF16, tag=f"kT{h}", name=f"kT{h}")
                   for h in range(8)]
            vaug = [pp.tile([128, 8, 65], BF16, tag=f"va{t}", name=f"va{t}")
                    for t in range(4)]
            s2one = pp.tile([128, 4, 512, 8], BF16, tag="s2one")  # [j', jt, i, h]
            mbt = [pp.tile([128, 512], BF16, tag=f"mb{t}", name=f"mb{t}")
                   for t in range(4)]
            attnU = [pp.tile([128, 8, 64, 8], BF16, tag=f"au{t}", name=f"au{t}")
                     for t in range(4)]                      # [j, h, t, g]
            o2sb = pp.tile([64, 512, 8], BF16, tag="o2sb")   # [d, i, h]
            oT8 = [pp.tile([64, 512], BF16, tag=f"oT{h}", name=f"oT{h}")
                   for h in range(8)]
            ones1 = pp.tile([1, 64], F32, tag="ones1")

            # loop-invariant constants
            nc.vector.memset(ones1[:], 1.0)
            for jt in range(4):
                nc.vector.memset(vaug[jt][:, :, 64], 1.0)

            def body():
                # ---- input DMAs
                for ck in range(4):
                    nc.sync.dma_start(out=wq_s[:, ck * 512:(ck + 1) * 512],
                                      in_=wqt[ck * 128:(ck + 1) * 128, :])
                for ck in range(4):
                    nc.sync.dma_start(out=xbq[:, ck * 512:(ck + 1) * 512],
                                      in_=qt[ck * 128:(ck + 1) * 128, :])
                nc.sync.dma_start(out=ident[:], in_=idn[:, :])
                for ck in range(4):
                    nc.sync.dma_start(out=wk_s[:, ck * 512:(ck + 1) * 512],
                                      in_=wkt[ck * 128:(ck + 1) * 128, :])
                    nc.sync.dma_start(out=xbk[:, ck * 512:(ck + 1) * 512],
                                      in_=kt[ck * 128:(ck + 1) * 128, :])
                for ck in range(4):
                    nc.sync.dma_start(out=wv_s[:, ck * 512:(ck + 1) * 512],
                                      in_=wvt[ck * 128:(ck + 1) * 128, :])
                    nc.sync.dma_start(out=xbv[:, ck * 512:(ck + 1) * 512],
                                      in_=vt[ck * 128:(ck + 1) * 128, :])
                for jt in range(4):
                    nc.sync.dma_start(out=mbt[jt][:], in_=mbd[jt, :, :])

                # ---- P1: projections (h-pair packed weight loads)
                scale = float(D ** -0.5)
                for hp in range(4):
                    pq = psA.tile([128, 512], F32, tag="psA")
                    for ck in range(4):
                        nc.tensor.matmul(
                            out=pq[:],
                            lhsT=wq_s[:, ck * 512 + hp * 128: ck * 512 + hp * 128 + 128],
                            rhs=xbq[:, ck * 512:(ck + 1) * 512],
                            start=(ck == 0), stop=(ck == 3))
                    for s in range(2):
                        h = 2 * hp + s
                        nc.scalar.activation(
                            out=qDT[:, h, :], in_=pq[s * 64:(s + 1) * 64, :],
                            func=mybir.ActivationFunctionType.Copy, scale=scale)
                        # q2 stacked pair columns (sum & diff), pre-scaled 0.5
                        q2v = q2[:].rearrange("p (pg sd h) -> p pg sd h",
                                              sd=2, h=8)
                        for par in range(2):
                            src = pq[s * 64:(s + 1) * 64, par::2]
                            for sd in range(2):
                                sgn = -1.0 if (par == 1 and sd == 1) else 1.0
                                nc.vector.tensor_scalar_mul(
                                    out=q2v[64 * par:64 * par + 64, :, sd, h],
                                    in0=src, scalar1=0.5 * scale * sgn)

                for hp in range(4):
                    pk = psA.tile([128, 512], F32, tag="psA")
                    for ck in range(4):
                        nc.tensor.matmul(
                            out=pk[:],
                            lhsT=wk_s[:, ck * 512 + hp * 128: ck * 512 + hp * 128 + 128],
                            rhs=xbk[:, ck * 512:(ck + 1) * 512],
                            start=(ck == 0), stop=(ck == 3))
                    for s in range(2):
                        h = 2 * hp + s
                        nc.scalar.activation(
                            out=kT8[h][:], in_=pk[s * 64:(s + 1) * 64, :],
                            func=mybir.ActivationFunctionType.Copy)

                for jt in range(4):  # v natural [j, hd]
                    pv = psA.tile([128, 512], F32, tag="psA")
                    for ck in range(4):
                        nc.tensor.matmul(
                            out=pv[:],
                            lhsT=xbv[:, ck * 512 + jt * 128: ck * 512 + (jt + 1) * 128],
                            rhs=wv_s[:, ck * 512:(ck + 1) * 512],
                            start=(ck == 0), stop=(ck == 3))
                    for h in range(8):
                        nc.vector.tensor_copy(out=vaug[jt][:, h, 0:64],
                                              in_=pv[:, h * 64:(h + 1) * 64])

                # ---- P2: score2 via K-packed sum/diff pair matmuls
                # ---- P3 (interleaved by i-half): qk + s2 + mask -> exp
                def p3_half(half):
                    lo = 256 * half
                    for jt in range(4):
                        for h in range(8):
                            pS = psB.tile([128, 256], F32, tag="psB")
                            nc.tensor.matmul(
                                out=pS[:],
                                lhsT=kT8[h][:, jt * 128:(jt + 1) * 128],
                                rhs=qDT[:, h, lo:lo + 256],
                                start=True, stop=False)
                            nc.tensor.matmul(
                                out=pS[:], lhsT=ident[:],
                                rhs=s2one[:, jt, lo:lo + 256, h],
                                start=False, stop=False)
                            nc.tensor.matmul(
                                out=pS[:], lhsT=ident[:],
                                rhs=mbt[jt][:, lo:lo + 256],
                                start=False, stop=True)
                            nc.scalar.activation(
                                out=attnU[jt][:, h,
                                              32 * half:32 * half + 32, :],
                                in_=pS[:],
                                func=mybir.ActivationFunctionType.Exp)

                for hb in list(range(8)) + ["p3a"] + list(range(8, 16)):
                    if hb == "p3a":
                        p3_half(0)
                        continue
                    abt = tsp.tile([128, 8192], FP8, tag="strm")
                    nc.sync.dma_start(out=abt[:], in_=ab2[hb, :, :, :])
                    psc = psC.tile([128, 4, 16, 2, 8], F32, tag="psC")
                    for jb in range(4):
                        for p in range(16):
                            pg = 16 * hb + p
                            nc.tensor.matmul(
                                out=psc[:, jb, p, :, :],
                                lhsT=abt[:, (p * 4 + jb) * 128:(p * 4 + jb + 1) * 128],
                                rhs=q2[:, pg * 16:(pg + 1) * 16],
                                start=True, stop=True)
                    # recombine all 4 jb at once: s0 = sum+diff, s1 = sum-diff
                    # (DVE reads at most one PSUM operand: stage diff)
                    dcp = stp.tile([128, 4, 16, 8], BF16, tag="dcp")
                    nc.vector.tensor_copy(out=dcp[:, :, :, :],
                                          in_=psc[:, :, :, 1, :])
                    dst = s2one[:, :, 32 * hb:32 * hb + 32, :].rearrange(
                        "p jb (q two) h -> p jb q two h", two=2)
                    nc.vector.tensor_add(out=dst[:, :, :, 0, :],
                                         in0=psc[:, :, :, 0, :],
                                         in1=dcp[:, :, :, :])
                    nc.vector.tensor_sub(out=dst[:, :, :, 1, :],
                                         in0=psc[:, :, :, 0, :],
                                         in1=dcp[:, :, :, :])

                p3_half(1)

                # ---- P4: bias-AV via M-packed pair fp8 matmuls
                for hb in range(16):
                    vtb = tsp.tile([128, 4, 16 * 128], FP8, tag="strm")
                    for jc in range(4):
                        nc.sync.dma_start(out=vtb[:, jc, :],
                                          in_=vtp[hb, jc, :, :])
                    po2 = psA.tile([128, 256], F32, tag="psA")
                    for p in range(16):
                        i0 = 32 * hb + 2 * p
                        tt, gg = i0 // 8, i0 % 8
                        for jc in range(4):
                            nc.tensor.matmul(
                                out=po2[:, p * 16:(p + 1) * 16],
                                lhsT=vtb[:, jc, p * 128:(p + 1) * 128],
                                rhs=attnU[jc][:, :, tt, gg:gg + 2],
                                start=(jc == 0), stop=(jc == 3))
                    src_e = po2[0:64].rearrange("p (q n) -> p q n", n=16)
                    src_o = po2[64:128].rearrange("p (q n) -> p q n", n=16)
                    dst = o2sb[:, 32 * hb:32 * hb + 32, :].rearrange(
                        "p (q two) h -> p q two h", two=2)
                    nc.vector.tensor_copy(
                        out=dst[:, :, 0, :],
                        in_=src_e.rearrange("p q (h two) -> p q two h",
                                            two=2)[:, :, 0, :])
                    nc.vector.tensor_copy(
                        out=dst[:, :, 1, :],
                        in_=src_o.rearrange("p q (h two) -> p q two h",
                                            two=2)[:, :, 1, :])

                # ---- P5: AV (+Z) and combine
                for hc in range(8):
                    nc.sync.dma_start(out=wo_s[:, hc * 512:(hc + 1) * 512],
                                      in_=wot[hc, :, :])
                for h in range(8):
                    po1 = psB.tile([65, 512], F32, tag="psB")
                    for jc in range(4):
                        nc.tensor.matmul(
                            out=po1[:],
                            lhsT=vaug[jc][:, h, :],
                            rhs=attnU[jc][:, h, :, :],
                            start=(jc == 0), stop=(jc == 3))
                    rz = stp.tile([1, 512], F32, tag="rz")
                    nc.vector.reciprocal(out=rz[:], in_=po1[64:65, :])
                    rzP = psA.tile([64, 512], F32, tag="psA")
                    nc.tensor.matmul(out=rzP[:], lhsT=ones1[:], rhs=rz[:],
                                     start=True, stop=True)
                    tmp = stp.tile([64, 512], F32, tag="cmb")
                    nc.vector.tensor_add(out=tmp[:], in0=po1[0:64, :],
                                         in1=o2sb[:, :, h])
                    nc.vector.tensor_mul(out=oT8[h][:], in0=tmp[:], in1=rzP[:])

                # ---- P6: output projection
                for it in range(4):
                    po = psA.tile([128, 512], F32, tag="psA")
                    for hc in range(8):
                        nc.tensor.matmul(out=po[:],
                                         lhsT=oT8[hc][:, it * 128:(it + 1) * 128],
                                         rhs=wo_s[:, hc * 512:(hc + 1) * 512],
                                         start=(hc == 0), stop=(hc == 7))
                    od = stp.tile([128, 512], F32, tag="od")
                    nc.vector.tensor_copy(out=od[:], in_=po[:])
                    nc.sync.dma_start(out=out[it * 128:(it + 1) * 128, :], in_=od[:])

            if loop_iters > 0:
                hint = (mybir.EngineType.PE, mybir.EngineType.DVE,
                        mybir.EngineType.Activation, mybir.EngineType.SP)
                with tc.For_i(0, loop_iters, 1, hint_engines=hint) as _:
                    body()
            else:
                body()

    nc.finalize()
    return nc


# ---------------------------------------------------------------- host side
def _host_prep(inputs):
    import ml_dtypes
    import concourse.mybir as mybir
    FP8NP = mybir.dt.np(mybir.dt.float8e4)
    BF16NP = ml_dtypes.bfloat16

    Q = np.asarray(inputs["Q"], np.float32)
    K = np.asarray(inputs["K"], np.float32)
    V = np.asarray(inputs["V"], np.float32)
    mask = np.asarray(inputs["mask"], bool)
    tb = np.asarray(inputs["time_bias"], np.int64)
    Wq = np.asarray(inputs["Wq"], np.float32)
    Wk = np.asarray(inputs["Wk"], np.float32)
    Wv = np.asarray(inputs["Wv"], np.float32)
    Wo = np.asarray(inputs["Wo"], np.float32)
    at = np.asarray(inputs["att_bias_tab"], np.float32)
    vt_tab = np.asarray(inputs["vec_bias_tab"], np.float32)

    wqt = np.ascontiguousarray(Wq.T).astype(BF16NP)
    wkt = np.ascontiguousarray(Wk.T).astype(BF16NP)
    wvt = np.ascontiguousarray(Wv.T).astype(BF16NP)
    wot = np.ascontiguousarray(Wo.T).astype(BF16NP).reshape(8, 64, 512)
    ident = np.eye(128, dtype=np.float32).astype(BF16NP)

    at8 = at.astype(FP8NP)                                   # [183, 64]
    vt8 = vt_tab.astype(FP8NP)

    in_maps = []
    for b in range(B):
        # ab2: [hb, (half,d), p, jb, j']
        Ga = at8[tb[b]]                                      # [512 i, 512 j, 64]
        arr = Ga.reshape(16, 16, 2, 4, 128, 64)              # hb p half jb j' d
        arr = arr.transpose(0, 2, 5, 1, 3, 4)                # hb half d p jb j'
        ab2 = np.ascontiguousarray(arr.reshape(16, 128, 2, 4096))
        # mbias
        mbd = np.where(mask[b].T, np.float32(MASK_NEG),
                       np.float32(0.0)).astype(BF16NP).reshape(4, 128, 512)
        # vtp pair-packed
        Gv = vt8[tb[b]]                                      # [512 i, 512 j, 64]
        Gv = np.ascontiguousarray(Gv.transpose(1, 0, 2))     # [j, i, d]
        vtp = Gv.reshape(4, 128, 16, 32, 64).transpose(2, 0, 1, 3, 4)
        vtp = np.ascontiguousarray(vtp.reshape(16, 4, 128, 16 * 128))
        in_maps.append({
            "qt": np.ascontiguousarray(Q[b].T).astype(BF16NP),
            "kt": np.ascontiguousarray(K[b].T).astype(BF16NP),
            "vt": np.ascontiguousarray(V[b].T).astype(BF16NP),
            "wqt": wqt, "wkt": wkt, "wvt": wvt, "wot": wot,
            "idn": ident, "ab2": ab2, "mbd": mbd,
            "vtp": vtp,
        })
    return in_maps


def kernel(**inputs):
    from concourse.bass_utils import run_bass_kernel_spmd
    key = "main"
    if key not in _NC_CACHE:
        _NC_CACHE[key] = build_kernel()
    nc = _NC_CACHE[key]
    in_maps = _host_prep(inputs)
    res = run_bass_kernel_spmd(nc, in_maps, core_ids=list(range(8)), trace=False)
    out = np.stack([res.results[b]["out"] for b in range(B)], axis=0)
    return out.astype(np.float32)
